# revision 1
# baseline (speedup 1.0000x reference)
"""Trainium2 Bass kernel for the argmax-distance-weighted loss.

loss = sum_b sum_{j,k} ((jstar_b - j)^2 + (kstar_b - k)^2) * t[b,j,k]
where (jstar_b, kstar_b) is the (first-occurrence) argmax location of t[b].

Decomposition used per batch b:
    loss_b = (jstar^2 + kstar^2)*S - 2*jstar*Sj - 2*kstar*Sk + Sj2 + Sk2
with S    = sum t[b]
     Sj   = sum_j j   * rowsum[b, j]      rowsum[b,j] = sum_k t[b,j,k]
     Sj2  = sum_j j^2 * rowsum[b, j]
     Sk   = sum_k k   * colsum[b, k]      colsum[b,k] = sum_j t[b,j,k]
     Sk2  = sum_k k^2 * colsum[b, k]

Device (8 NeuronCores, data-parallel over batch, 8x [128, 64, 64] tiles per
core): per tile the DVE runs two single-source tensor_reduce passes (rowsum,
rowmax) on its dedicated SBUF port while GpSimd computes colsum concurrently
as a contiguous-halves fold tree (out[i] = in[i] + in[i+half], which folds j
away and keeps k innermost) on the DVE/GpSimd shared port. A batched
epilogue derives M, S, Sj, Sj2, Sk, Sk2 and rj per batch; jstar = 64 - rj is
exact (first row whose rowmax equals the batch max, via an is_ge mask times
a reversed-index weight, max-reduced). Engines balance at ~70us (DVE) /
~32us (Pool) / ~53us (DMA) per core; steady-state ~72us, single-shot ~84us
vs a ~47us HBM roofline (16.8 MB/core at ~358 GB/s).

Host (the "gather/unshard" step): gathers row jstar per batch (64 floats,
0.4% of the data) to resolve kstar with exact first-occurrence tie
semantics — matching jnp.argmax's flat scan order exactly, since the first
flat maximum is (first row containing M, first k with M within that row) —
then evaluates the closed form in float64 and sums. The tie handling is
load-bearing: the actual jax.random input has batches with duplicated
maxima.

Toolchain notes (this container's pinned walrus build):
- only ONE sync-wait is encodable per TPB instruction; Tile attaches
  several (tail drain, DMA copies) -> _split_multiwait_instructions
  post-pass hoists extras into standalone same-engine NoOps.
- InstTensorTensorReduce ("ISA wrong length"), InstPool (verifier assert),
  and TensorScalarPtr-on-Pool ("engine check failed") are all unusable;
  Pool accepts only arithmetic InstTensorTensor (add/mult, no is_ge).
- GpSimd throughput: contiguous tensor_tensor streams fast (~0.7 ns/elem),
  strided APs are several times slower on real HW than the cost model says.
"""

import os
import sys

import numpy as np

try:
    import concourse.bass as bass
except ModuleNotFoundError:  # make concourse importable in a bare container
    for _p in ("/opt/trn_rl_repo", "/root/.axon_site/_ro/trn_rl_repo"):
        if os.path.isdir(_p) and _p not in sys.path:
            sys.path.insert(0, _p)
    import concourse.bass as bass

import concourse.mybir as mybir
from concourse.bass_utils import run_bass_kernel_spmd
from concourse.tile import TileContext
# --- workaround: this walrus build encodes only ONE sync-wait per TPB ---
# instruction. Tile attaches several waits to one instruction (tail drain,
# DMA copies, ...), which codegen rejects with "Too many sync wait
# commands". Post-pass: hoist all but the last wait of each instruction
# into standalone same-engine NoOps placed immediately before it.


def _split_multiwait_instructions(nc: bass.Bass) -> None:
    # (bb, inst-name) pairs needing surgery
    targets = []
    for fn in nc.m.functions:
        for bb in fn.blocks:
            for inst in bb.instructions:
                si = inst.sync_info
                if si is not None and len(si.on_wait) > 1:
                    targets.append((bb, inst.name))
    if not targets:
        return

    moved_nop_names: set[str] = set()
    plan: dict[str, list] = {}  # target-inst-name -> nop instructions
    for bb, iname in targets:
        inst = next(i for i in bb.instructions if i.name == iname)
        waits = list(inst.sync_info.on_wait)
        inst.sync_info.on_wait = waits[-1:]
        nops = []
        for w in waits[:-1]:
            bi = nc.engines[inst.engine].nop(nofuse=True, hint="split_wait")
            bi.ins.sync_info = mybir.SyncInfo(on_wait=[w], on_update=[])
            nops.append(bi.ins)
            moved_nop_names.add(bi.ins.name)
        plan[iname] = nops

    # relocate the nops to sit immediately before their target instruction
    for fn in nc.m.functions:
        for bb in fn.blocks:
            insts = list(bb.instructions)
            kept = [i for i in insts if i.name not in moved_nop_names]
            out: list = []
            changed = len(kept) != len(insts)
            for inst in kept:
                if inst.name in plan:
                    out.extend(plan[inst.name])
                    changed = True
                out.append(inst)
            if changed:
                bb.instructions = out

B, H, W = 8192, 64, 64
NCORES = 8
P = 128  # SBUF partitions

F32 = mybir.dt.float32
Alu = mybir.AluOpType
Ax = mybir.AxisListType

# output layout: quantity-major [P, NQ, ntiles]
Q_M, Q_S, Q_SJ, Q_SJ2, Q_SK, Q_SK2, Q_RJ = range(7)
NQ = 7


def build(bpc: int, repeats: int = 1, gp: bool = True, gp_rs: int = 0) -> bass.Bass:
    """Build the per-core Bass program for `bpc` batches per core.

    `repeats` re-runs the whole pipeline N times in one program — used only
    for timing (slope method cancels the host dispatch overhead).
    `gp` offloads the colsum fold tree and the elementwise muls to GpSimd so
    the DVE runs almost only dedicated-port single-src reductions."""
    ntiles = bpc // P
    assert ntiles * P == bpc
    NT = ntiles

    nc = bass.Bass()
    x = nc.declare_dram_parameter("x", [bpc, H, W], F32, isOutput=False)
    wc = nc.declare_dram_parameter("wconsts", [3, NT, W], F32, isOutput=False)
    wf = nc.declare_dram_parameter("wfull", [H * W], F32, isOutput=False)
    out = nc.declare_dram_parameter("moments", [P, NQ * NT], F32, isOutput=True)

    with TileContext(nc) as tc:
        with (
            tc.tile_pool(name="xpool", bufs=4) as xpool,
            tc.tile_pool(name="folds", bufs=2) as fpool,
            tc.tile_pool(name="consts", bufs=1) as cpool,
            tc.tile_pool(name="inter", bufs=1) as ipool,
        ):
            # broadcast weight constants [3, NT, W] across all partitions
            wtile = cpool.tile([P, 3, NT, W], F32)
            wc_ap = wc[:, :, :]
            bcast = bass.AP(
                tensor=wc_ap.tensor,
                offset=wc_ap.offset,
                ap=[[0, P]] + list(wc_ap.ap),
            )
            nc.scalar.dma_start(out=wtile, in_=bcast)
            wftile = cpool.tile([P, H * W], F32)
            wf_ap = wf[:]
            nc.scalar.dma_start(out=wftile, in_=bass.AP(
                tensor=wf_ap.tensor, offset=wf_ap.offset,
                ap=[[0, P]] + list(wf_ap.ap)))
            w1 = wtile[:, 0, :, :]  # [P, NT, W] = j (0..63), tiled per tile
            wr = wtile[:, 2, :, :]  # [P, NT, W] = 64-j

            rs_all = ipool.tile([P, NT, W], F32)
            cs_all = ipool.tile([P, NT, W], F32)
            rm_all = ipool.tile([P, NT, W], F32)
            scrA = ipool.tile([P, NT, W], F32)
            scrB = ipool.tile([P, NT, W], F32)
            scrC = ipool.tile([P, NT, W], F32)
            scrD = ipool.tile([P, NT, W], F32)
            outq = ipool.tile([P, NQ * NT], F32)

            def O(q):
                return outq[:, q * NT : (q + 1) * NT]

            eng = nc.gpsimd if gp else nc.vector

            for rep in range(repeats):
                for t in range(ntiles):
                    xt = xpool.tile([P, H, W], F32)
                    nchunk = {0: 4, 1: 2, 2: 2}.get(t, 1) if repeats == 1 else (4 if t == 0 else 1)
                    hs = H // nchunk
                    for c in range(nchunk):
                        nc.sync.dma_start(
                            out=xt[:, c * hs : (c + 1) * hs, :],
                            in_=x[t * P : (t + 1) * P, c * hs : (c + 1) * hs, :],
                        )

                    # rowmax always on DVE (single-src, dedicated port)
                    for c in range(nchunk):
                        nc.vector.tensor_reduce(
                            out=rm_all[:, t, c * hs : (c + 1) * hs],
                            in_=xt[:, c * hs : (c + 1) * hs, :], axis=Ax.X, op=Alu.max,
                        )
                    act_tile = False  # ACT offload: sim says 73us but HW says 139 - cost model blind spot
                    if not act_tile:
                        for c in range(nchunk):
                            nc.vector.tensor_reduce(
                                out=rs_all[:, t, c * hs : (c + 1) * hs],
                                in_=xt[:, c * hs : (c + 1) * hs, :], axis=Ax.X, op=Alu.add,
                            )
                    else:
                        # j-moments without a DVE rowsum pass: GP multiplies
                        # by the full j-weight tile (contiguous), the idle
                        # ACT engine accumulates Sj / Sj2 directly.
                        xw = fpool.tile([P, H * W], F32, tag="xw")
                        xf0 = xt[:, :, :].rearrange("p a b -> p (a b)")
                        nc.gpsimd.tensor_tensor(out=xw, in0=xf0, in1=wftile, op=Alu.mult)
                        nc.scalar.activation(
                            out=xw, in_=xw, func=mybir.ActivationFunctionType.Copy,
                            accum_out=outq[:, Q_SJ * NT + t : Q_SJ * NT + t + 1],
                        )
                        xw2 = fpool.tile([P, H * W], F32, tag="xw2")
                        nc.gpsimd.tensor_tensor(out=xw2, in0=xw, in1=wftile, op=Alu.mult)
                        nc.scalar.activation(
                            out=xw2, in_=xw2, func=mybir.ActivationFunctionType.Copy,
                            accum_out=outq[:, Q_SJ2 * NT + t : Q_SJ2 * NT + t + 1],
                        )
                    if gp:
                        # colsum over j as a fold tree on flat contiguous
                        # halves: out[i] = in[i] + in[i+half] (k stays the
                        # innermost 64)
                        xf = xt[:, :, :].rearrange("p a b -> p (a b)")
                        fold = fpool.tile([P, 4096], F32, tag="fold")
                        if t == 0:
                            # two half-width L0 folds so GP starts after the
                            # first two DMA chunks instead of all four
                            nc.gpsimd.tensor_tensor(
                                out=fold[:, 0:1024], in0=xf[:, 0:1024],
                                in1=xf[:, 2048:3072], op=Alu.add,
                            )
                            nc.gpsimd.tensor_tensor(
                                out=fold[:, 1024:2048], in0=xf[:, 1024:2048],
                                in1=xf[:, 3072:4096], op=Alu.add,
                            )
                            seg = [(2048, 1024), (3072, 512),
                                   (3584, 256), (3840, 128)]
                            src, src_off = fold, 0
                        else:
                            seg = [(0, 2048), (2048, 1024), (3072, 512),
                                   (3584, 256), (3840, 128)]
                            src, src_off = xf, 0
                        for (dst_off, dst_n) in seg:
                            nc.gpsimd.tensor_tensor(
                                out=fold[:, dst_off : dst_off + dst_n],
                                in0=src[:, src_off : src_off + dst_n],
                                in1=src[:, src_off + dst_n : src_off + 2 * dst_n],
                                op=Alu.add,
                            )
                            src, src_off = fold, dst_off
                        nc.gpsimd.tensor_tensor(
                            out=cs_all[:, t, :], in0=fold[:, 3840:3904],
                            in1=fold[:, 3904:3968], op=Alu.add,
                        )
                    else:
                        xk = xt[:, :, :].rearrange("p j k -> p k j")
                        nc.vector.tensor_reduce(
                            out=cs_all[:, t, :], in_=xk, axis=Ax.X, op=Alu.add
                        )

                    # weighted products per tile, off the critical tail
                    if not act_tile:
                        eng.tensor_tensor(out=scrA[:, t, :], in0=rs_all[:, t, :],
                                          in1=w1[:, t, :], op=Alu.mult)
                        eng.tensor_tensor(out=scrB[:, t, :], in0=scrA[:, t, :],
                                          in1=w1[:, t, :], op=Alu.mult)
                        nc.vector.tensor_reduce(
                            out=outq[:, Q_SJ * NT + t : Q_SJ * NT + t + 1],
                            in_=scrA[:, t, :], axis=Ax.X, op=Alu.add)
                        nc.vector.tensor_reduce(
                            out=outq[:, Q_SJ2 * NT + t : Q_SJ2 * NT + t + 1],
                            in_=scrB[:, t, :], axis=Ax.X, op=Alu.add)
                    eng.tensor_tensor(out=scrC[:, t, :], in0=cs_all[:, t, :],
                                      in1=w1[:, t, :], op=Alu.mult)
                    eng.tensor_tensor(out=scrD[:, t, :], in0=scrC[:, t, :],
                                      in1=w1[:, t, :], op=Alu.mult)

                # epilogue: plain reductions (SJ/SJ2 written per tile)
                nc.vector.tensor_reduce(out=O(Q_SK), in_=scrC[:, :, :], axis=Ax.X, op=Alu.add)
                nc.vector.tensor_reduce(out=O(Q_SK2), in_=scrD[:, :, :], axis=Ax.X, op=Alu.add)
                nc.vector.tensor_reduce(out=O(Q_S), in_=cs_all[:, :, :], axis=Ax.X, op=Alu.add)
                nc.vector.tensor_reduce(out=O(Q_M), in_=rm_all[:, :, :], axis=Ax.X, op=Alu.max)
                # jstar: ge = (rm >= M) * (64-j); rj = max; jstar = 64 - rj
                mb = O(Q_M).unsqueeze(2).to_broadcast([P, NT, W])
                nc.vector.tensor_tensor(out=scrB, in0=rm_all, in1=mb, op=Alu.is_ge)
                eng.tensor_tensor(out=scrA, in0=scrB, in1=wr, op=Alu.mult)
                nc.vector.tensor_reduce(out=O(Q_RJ), in_=scrA[:, :, :], axis=Ax.X, op=Alu.max)

            nc.sync.dma_start(out=out[:, :], in_=outq)

    _split_multiwait_instructions(nc)
    return nc


_cache: dict[int, bass.Bass] = {}


def _get(bpc: int) -> bass.Bass:
    if bpc not in _cache:
        _cache[bpc] = build(bpc)
    return _cache[bpc]


def _wconsts(ntiles: int) -> np.ndarray:
    j = np.arange(W, dtype=np.float32)
    base = np.stack([j, j * j, (W - j).astype(np.float32)])  # [3, W]
    return np.repeat(base[:, None, :], ntiles, axis=1)  # [3, NT, W]


def _prepare(tensor: np.ndarray):
    t = np.ascontiguousarray(np.asarray(tensor), dtype=np.float32)
    bt = t.shape[0]
    bpc = bt // NCORES
    nc = _get(bpc)
    wc = _wconsts(bpc // P)
    wfull = np.repeat(np.arange(H, dtype=np.float32), W)
    in_maps = [
        {"x": t[c * bpc : (c + 1) * bpc], "wconsts": wc, "wfull": wfull}
        for c in range(NCORES)
    ]
    return nc, in_maps, t


def _postprocess(t: np.ndarray, results: list[dict]) -> np.ndarray:
    bt = t.shape[0]
    bpc = bt // NCORES
    nt = bpc // P
    ms = []
    for c in range(NCORES):
        m = results[c]["moments"].reshape(P, NQ, nt)
        ms.append(m.transpose(2, 0, 1).reshape(bpc, NQ))  # batch-major
    m = np.concatenate(ms, 0).astype(np.float64)  # [B, NQ]

    S = m[:, Q_S]
    Sj = m[:, Q_SJ]
    Sj2 = m[:, Q_SJ2]
    Sk = m[:, Q_SK]
    Sk2 = m[:, Q_SK2]
    jstar = np.rint(W - m[:, Q_RJ]).astype(np.int64)

    # resolve kstar with exact first-occurrence semantics on the argmax row
    rows = t[np.arange(bt), jstar, :]  # [B, W]
    mrow = rows.max(axis=1)
    kstar = (rows == mrow[:, None]).argmax(axis=1)

    js = jstar.astype(np.float64)
    ks = kstar.astype(np.float64)
    loss = ((js * js + ks * ks) * S - 2.0 * js * Sj - 2.0 * ks * Sk + Sj2 + Sk2).sum()
    return np.asarray([loss], dtype=np.float32)


def kernel(tensor: np.ndarray) -> np.ndarray:
    nc, in_maps, t = _prepare(tensor)
    res = run_bass_kernel_spmd(nc, in_maps, list(range(NCORES)))
    return _postprocess(t, res.results)



# revision 2
# speedup vs baseline: 1.5178x; 1.5178x over previous
"""Trainium2 Bass kernel v2 (fp16 variant) for the argmax-distance-weighted loss.

loss = sum_b sum_{j,k} ((jstar_b - j)^2 + (kstar_b - k)^2) * t[b,j,k]
with (jstar_b, kstar_b) the first-occurrence argmax location of t[b].

Decomposition per batch:
    loss_b = (js^2 + ks^2)*S - 2*js*Sj - 2*ks*Sk + Sj2 + Sk2
    S   = sum t[b]      Sk  = sum_k k  * colsum[b,k]   Sk2 = sum_k k^2 * colsum
    Sj  = sum_j j * rowsum[b,j]        Sj2 = sum_j j^2 * rowsum

Device architecture (8 cores, data-parallel over batch, 8 x [128,64,64]
tiles per core):
  - GpSimd issues SWDGE casting DMAs: f32 HBM -> fp16 SBUF (HBM read traffic
    unchanged; SBUF data 2-byte so the DVE runs its folds in 2x mode).
  - DVE computes rowsum and rowmax per tile as contiguous-halves fold trees
    over k using fp16 tensor_tensor (2x_1p mode, ~0.52 ns/elem vs 1.04 for
    tensor_reduce, which has no fast dtype mode at all).
  - PE computes the colsum family: 32 accumulated transpose-via-identity
    matmuls per tile (lhsT = x chunk [128b, 128f], rhs = fp16 identity ->
    P[f,b] += x.T in f32 PSUM), then a second matmul with stationary
    {1, k, k^2} over the transposed partitions (j-parity, k) -> S/Sk/Sk2 per
    batch. All accumulation is f32, so only the input fp16 cast (~2^-11)
    contributes error.
  - Outputs: mom [3, 1024] f32, rowsums [128, NT*64] f32, rowmaxes fp16.

Host resolves jstar/kstar with exact first-occurrence flat-argmax semantics:
fp16 rowmax is monotone, so the true argmax row is always in the candidate
set {j : rm_fp16[j] == max(rm_fp16)}; candidates are re-scored with f32 row
maxima gathered from the original input (a few rows per batch), then Sj/Sj2
come from the f32 rowsums and the closed form is evaluated in f64.

Measured (8-core SPMD, slope method): ~35-44 us steady-state vs ~93 us for
the previous DVE-reduce baseline; rel err ~1.4e-6.

Toolchain notes (pinned walrus build): only ONE sync-wait encodable per
instruction -> _split_multiwait_instructions post-pass; InstTensorTensorReduce
/ InstPool / TensorScalarPtr-on-Pool are unusable; DMA from HWDGE queues is
sync (SP) and scalar (ACT) only; casting DMA requires gpsimd SWDGE.
"""

import os
import sys

import numpy as np

try:
    import concourse.bass as bass
except ModuleNotFoundError:
    for _p in ("/opt/trn_rl_repo", "/root/.axon_site/_ro/trn_rl_repo"):
        if os.path.isdir(_p) and _p not in sys.path:
            sys.path.insert(0, _p)
    import concourse.bass as bass

import ml_dtypes
import concourse.mybir as mybir
from concourse.bass_utils import run_bass_kernel_spmd
from concourse.tile import TileContext

B, H, W = 8192, 64, 64
NCORES = 8
P = 128

F32 = mybir.dt.float32
F16 = mybir.dt.float16  # 2-byte: enables DVE 2x_1p; 10-bit mantissa
Alu = mybir.AluOpType
Ax = mybir.AxisListType
ActF = mybir.ActivationFunctionType


def _split_multiwait_instructions(nc: bass.Bass) -> None:
    """Hoist all but the last sync-wait of each instruction into standalone
    same-engine NoOps (this walrus build encodes only one wait per TPB)."""
    targets = []
    for fn in nc.m.functions:
        for bb in fn.blocks:
            for inst in bb.instructions:
                si = inst.sync_info
                if si is not None and len(si.on_wait) > 1:
                    targets.append((bb, inst.name))
    if not targets:
        return

    moved_nop_names: set[str] = set()
    plan: dict[str, list] = {}
    for bb, iname in targets:
        inst = next(i for i in bb.instructions if i.name == iname)
        waits = list(inst.sync_info.on_wait)
        inst.sync_info.on_wait = waits[-1:]
        nops = []
        for w in waits[:-1]:
            bi = nc.engines[inst.engine].nop(nofuse=True, hint="split_wait")
            bi.ins.sync_info = mybir.SyncInfo(on_wait=[w], on_update=[])
            nops.append(bi.ins)
            moved_nop_names.add(bi.ins.name)
        plan[iname] = nops

    for fn in nc.m.functions:
        for bb in fn.blocks:
            insts = list(bb.instructions)
            kept = [i for i in insts if i.name not in moved_nop_names]
            out: list = []
            changed = len(kept) != len(insts)
            for inst in kept:
                if inst.name in plan:
                    out.extend(plan[inst.name])
                    changed = True
                out.append(inst)
            if changed:
                bb.instructions = out


def build(bpc: int, repeats: int = 1) -> bass.Bass:
    """Per-core program for `bpc` batches. `repeats` re-runs the pipeline
    (timing only; slope method cancels dispatch overhead)."""
    NT = bpc // P
    assert NT * P == bpc

    nc = bass.Bass()
    x = nc.declare_dram_parameter("x", [bpc, H, W], F32, isOutput=False)
    ident_d = nc.declare_dram_parameter("ident", [P, P], F16, isOutput=False)
    w3_d = nc.declare_dram_parameter("w3", [P, 3], F32, isOutput=False)
    mom_d = nc.declare_dram_parameter("mom", [3, bpc], F32, isOutput=True)
    rs_d = nc.declare_dram_parameter("rs", [P, NT * H], F32, isOutput=True)
    rm_d = nc.declare_dram_parameter("rm", [P, NT * H], F16, isOutput=True)

    with TileContext(nc) as tc:
        with (
            tc.tile_pool(name="xpool", bufs=3) as xpool,
            tc.tile_pool(name="fpool", bufs=2) as fpool,
            tc.tile_pool(name="cpool", bufs=1) as cpool,
            tc.tile_pool(name="opool", bufs=1) as opool,
            tc.tile_pool(name="apool", bufs=2) as apool,
            tc.psum_pool(name="psP", bufs=2) as psP,
            tc.psum_pool(name="psQ", bufs=2) as psQ,
        ):
            ident = cpool.tile([P, P], F16)
            nc.sync.dma_start(out=ident, in_=ident_d[:, :])
            w3 = cpool.tile([P, 3], F32)
            nc.sync.dma_start(out=w3, in_=w3_d[:, :])

            rs_sb = opool.tile([P, NT, H], F32)
            rm_sb = opool.tile([P, NT, H], F16)
            mom_sb = opool.tile([3, NT, P], F32)

            for rep in range(repeats):
                pend = []  # (tile_idx, P_psum, Asb)
                for t in range(NT):
                    xb = xpool.tile([P, H, W], F16, tag="x")
                    nchunk = 2 if t == 0 else 1
                    hs = H // nchunk
                    for c in range(nchunk):
                        nc.gpsimd.dma_start(
                            out=xb[:, c * hs : (c + 1) * hs, :],
                            in_=x[t * P : (t + 1) * P, c * hs : (c + 1) * hs, :],
                        )
                    xf = xb.rearrange("p a b -> p (a b)")

                    # --- DVE rowsum fold over k (bf16 2x mode) ---
                    sc = fpool.tile([P, H, W // 2], F16, tag="sc")
                    nc.vector.tensor_tensor(
                        out=sc, in0=xb[:, :, 0:32], in1=xb[:, :, 32:64], op=Alu.add
                    )
                    w = W // 4
                    while w >= 2:
                        nc.vector.tensor_tensor(
                            out=sc[:, :, 0:w], in0=sc[:, :, 0:w],
                            in1=sc[:, :, w : 2 * w], op=Alu.add,
                        )
                        w //= 2
                    nc.vector.tensor_tensor(
                        out=rs_sb[:, t, :].unsqueeze(2), in0=sc[:, :, 0:1],
                        in1=sc[:, :, 1:2], op=Alu.add,
                    )

                    # --- DVE rowmax fold over k (bf16, exact) ---
                    mc = fpool.tile([P, H, W // 2], F16, tag="mc")
                    nc.vector.tensor_tensor(
                        out=mc, in0=xb[:, :, 0:32], in1=xb[:, :, 32:64], op=Alu.max
                    )
                    w = W // 4
                    while w >= 2:
                        nc.vector.tensor_tensor(
                            out=mc[:, :, 0:w], in0=mc[:, :, 0:w],
                            in1=mc[:, :, w : 2 * w], op=Alu.max,
                        )
                        w //= 2
                    nc.vector.tensor_tensor(
                        out=rm_sb[:, t, :].unsqueeze(2), in0=mc[:, :, 0:1],
                        in1=mc[:, :, 1:2], op=Alu.max,
                    )

                    # --- PE colsum pyramid: Pt[(j', k), b] += chunk.T ---
                    Pt = psP.tile([P, P], F32, tag="P")
                    for c in range(H * W // P):
                        nc.tensor.matmul(
                            out=Pt, lhsT=xf[:, c * P : (c + 1) * P], rhs=ident,
                            start=(c == 0), stop=(c == H * W // P - 1),
                        )
                    Asb = apool.tile([P, P], F32, tag="A")
                    nc.scalar.activation(out=Asb, in_=Pt, func=ActF.Copy)

                    # stage-2 for the previous tile keeps PE from stalling on
                    # the ACT drain of this tile's pyramid
                    if pend:
                        tp, Asb_p = pend.pop()
                        Qt = psQ.tile([3, P], F32, tag="Q")
                        nc.tensor.matmul(out=Qt, lhsT=w3, rhs=Asb_p,
                                         start=True, stop=True)
                        nc.scalar.activation(
                            out=mom_sb[:, tp, :], in_=Qt, func=ActF.Copy)
                    pend.append((t, Asb))

                tp, Asb_p = pend.pop()
                Qt = psQ.tile([3, P], F32, tag="Q")
                nc.tensor.matmul(out=Qt, lhsT=w3, rhs=Asb_p, start=True, stop=True)
                nc.scalar.activation(out=mom_sb[:, tp, :], in_=Qt, func=ActF.Copy)

            nc.sync.dma_start(out=mom_d[:, :], in_=mom_sb.rearrange("a b c -> a (b c)"))
            nc.sync.dma_start(out=rs_d[:, :], in_=rs_sb.rearrange("p a b -> p (a b)"))
            nc.sync.dma_start(out=rm_d[:, :], in_=rm_sb.rearrange("p a b -> p (a b)"))

    _split_multiwait_instructions(nc)
    return nc


_cache: dict[int, bass.Bass] = {}


def _get(bpc: int) -> bass.Bass:
    if bpc not in _cache:
        _cache[bpc] = build(bpc)
    return _cache[bpc]


def _consts():
    ident = np.eye(P, dtype=np.float16)
    k = (np.arange(P) % W).astype(np.float32)
    w3 = np.stack([np.ones(P, np.float32), k, k * k], axis=1)  # [128, 3]
    return ident, w3


def _prepare(tensor: np.ndarray):
    t = np.ascontiguousarray(np.asarray(tensor), dtype=np.float32)
    bt = t.shape[0]
    bpc = bt // NCORES
    nc = _get(bpc)
    ident, w3 = _consts()
    in_maps = [
        {"x": t[c * bpc : (c + 1) * bpc], "ident": ident, "w3": w3}
        for c in range(NCORES)
    ]
    return nc, in_maps, t


def _postprocess(t: np.ndarray, results: list[dict]) -> np.ndarray:
    bt = t.shape[0]
    bpc = bt // NCORES
    nt = bpc // P

    mom = np.concatenate(
        [r["mom"].reshape(3, bpc) for r in results], axis=1
    ).astype(np.float64)  # [3, B] batch index = c*bpc + t*128 + p
    rs = np.concatenate(
        [r["rs"].reshape(P, nt, H).transpose(1, 0, 2).reshape(bpc, H)
         for r in results], axis=0)  # [B, H] f32, b = c*bpc + t*128 + p
    rm = np.concatenate(
        [r["rm"].reshape(P, nt, H).transpose(1, 0, 2).reshape(bpc, H)
         for r in results], axis=0).astype(np.float32)

    S, Sk, Sk2 = mom[0], mom[1], mom[2]
    j = np.arange(H, dtype=np.float64)
    Sj = rs.astype(np.float64) @ j
    Sj2 = rs.astype(np.float64) @ (j * j)

    # exact first-occurrence argmax: candidates are rows whose bf16 rowmax
    # ties the bf16 batch max (monotone cast -> true argmax row included)
    Mb = rm.max(axis=1)
    bidx, jidx = np.nonzero(rm == Mb[:, None])
    key = t[bidx, jidx, :].max(axis=1)  # f32 row maxima of candidates
    order = np.lexsort((jidx, -key, bidx))  # per batch: max key, then min j
    first = np.searchsorted(bidx[order], np.arange(bt))
    jstar = jidx[order][first]

    rows = t[np.arange(bt), jstar, :]
    kstar = (rows == rows.max(axis=1)[:, None]).argmax(axis=1)

    js = jstar.astype(np.float64)
    ks = kstar.astype(np.float64)
    loss = ((js * js + ks * ks) * S - 2.0 * js * Sj - 2.0 * ks * Sk + Sj2 + Sk2).sum()
    return np.asarray([loss], dtype=np.float32)


def kernel(tensor: np.ndarray) -> np.ndarray:
    nc, in_maps, t = _prepare(tensor)
    res = run_bass_kernel_spmd(nc, in_maps, list(range(NCORES)))
    return _postprocess(t, res.results)


# revision 3
# speedup vs baseline: 1.6927x; 1.1152x over previous
"""Trainium2 Bass kernel v2 (fp16 variant) for the argmax-distance-weighted loss.

loss = sum_b sum_{j,k} ((jstar_b - j)^2 + (kstar_b - k)^2) * t[b,j,k]
with (jstar_b, kstar_b) the first-occurrence argmax location of t[b].

Decomposition per batch:
    loss_b = (js^2 + ks^2)*S - 2*js*Sj - 2*ks*Sk + Sj2 + Sk2
    S   = sum t[b]      Sk  = sum_k k  * colsum[b,k]   Sk2 = sum_k k^2 * colsum
    Sj  = sum_j j * rowsum[b,j]        Sj2 = sum_j j^2 * rowsum

Device architecture (8 cores, data-parallel over batch, 8 x [128,64,64]
tiles per core):
  - GpSimd issues SWDGE casting DMAs: f32 HBM -> fp16 SBUF (HBM read traffic
    unchanged; SBUF data 2-byte so the DVE runs its folds in 2x mode).
  - DVE computes rowsum and rowmax per tile as contiguous-halves fold trees
    over k using fp16 tensor_tensor (2x_1p mode, ~0.52 ns/elem vs 1.04 for
    tensor_reduce, which has no fast dtype mode at all).
  - PE computes the colsum family: 32 accumulated transpose-via-identity
    matmuls per tile (lhsT = x chunk [128b, 128f], rhs = fp16 identity ->
    P[f,b] += x.T in f32 PSUM), then a second matmul with stationary
    {1, k, k^2} over the transposed partitions (j-parity, k) -> S/Sk/Sk2 per
    batch. All accumulation is f32, so only the input fp16 cast (~2^-11)
    contributes error.
  - Outputs: mom [3, 1024] f32, rowsums [128, NT*64] f32, rowmaxes fp16.

Host resolves jstar/kstar with exact first-occurrence flat-argmax semantics:
fp16 rowmax is monotone, so the true argmax row is always in the candidate
set {j : rm_fp16[j] == max(rm_fp16)}; candidates are re-scored with f32 row
maxima gathered from the original input (a few rows per batch), then Sj/Sj2
come from the f32 rowsums and the closed form is evaluated in f64.

Measured (8-core SPMD, slope method): ~35-44 us steady-state vs ~93 us for
the previous DVE-reduce baseline; rel err ~1.4e-6.

Toolchain notes (pinned walrus build): only ONE sync-wait encodable per
instruction -> _split_multiwait_instructions post-pass; InstTensorTensorReduce
/ InstPool / TensorScalarPtr-on-Pool are unusable; DMA from HWDGE queues is
sync (SP) and scalar (ACT) only; casting DMA requires gpsimd SWDGE.
"""

import os
import sys

import numpy as np

try:
    import concourse.bass as bass
except ModuleNotFoundError:
    for _p in ("/opt/trn_rl_repo", "/root/.axon_site/_ro/trn_rl_repo"):
        if os.path.isdir(_p) and _p not in sys.path:
            sys.path.insert(0, _p)
    import concourse.bass as bass

import ml_dtypes
import concourse.mybir as mybir
from concourse.bass_utils import run_bass_kernel_spmd
from concourse.tile import TileContext

B, H, W = 8192, 64, 64
NCORES = 8
P = 128

F32 = mybir.dt.float32
F16 = mybir.dt.float16  # 2-byte: enables DVE 2x_1p; 10-bit mantissa
Alu = mybir.AluOpType
Ax = mybir.AxisListType
ActF = mybir.ActivationFunctionType


def _split_multiwait_instructions(nc: bass.Bass) -> None:
    """Hoist all but the last sync-wait of each instruction into standalone
    same-engine NoOps (this walrus build encodes only one wait per TPB)."""
    targets = []
    for fn in nc.m.functions:
        for bb in fn.blocks:
            for inst in bb.instructions:
                si = inst.sync_info
                if si is not None and len(si.on_wait) > 1:
                    targets.append((bb, inst.name))
    if not targets:
        return

    moved_nop_names: set[str] = set()
    plan: dict[str, list] = {}
    for bb, iname in targets:
        inst = next(i for i in bb.instructions if i.name == iname)
        waits = list(inst.sync_info.on_wait)
        inst.sync_info.on_wait = waits[-1:]
        nops = []
        for w in waits[:-1]:
            bi = nc.engines[inst.engine].nop(nofuse=True, hint="split_wait")
            bi.ins.sync_info = mybir.SyncInfo(on_wait=[w], on_update=[])
            nops.append(bi.ins)
            moved_nop_names.add(bi.ins.name)
        plan[iname] = nops

    for fn in nc.m.functions:
        for bb in fn.blocks:
            insts = list(bb.instructions)
            kept = [i for i in insts if i.name not in moved_nop_names]
            out: list = []
            changed = len(kept) != len(insts)
            for inst in kept:
                if inst.name in plan:
                    out.extend(plan[inst.name])
                    changed = True
                out.append(inst)
            if changed:
                bb.instructions = out


def build(bpc: int, repeats: int = 1) -> bass.Bass:
    """Per-core program for `bpc` batches. `repeats` re-runs the pipeline
    (timing only; slope method cancels dispatch overhead)."""
    NT = bpc // P
    assert NT * P == bpc

    nc = bass.Bass()
    x = nc.declare_dram_parameter("x", [bpc, H, W], F32, isOutput=False)
    ident_d = nc.declare_dram_parameter("ident", [P, P], F16, isOutput=False)
    w3_d = nc.declare_dram_parameter("w3", [P, 3], F32, isOutput=False)
    mom_d = nc.declare_dram_parameter("mom", [3, bpc], F32, isOutput=True)
    rs_d = nc.declare_dram_parameter("rs", [P, NT * H], F32, isOutput=True)
    rm_d = nc.declare_dram_parameter("rm", [P, NT * H], F16, isOutput=True)

    with TileContext(nc) as tc:
        with (
            tc.tile_pool(name="xpool", bufs=3) as xpool,
            tc.tile_pool(name="fpool", bufs=2) as fpool,
            tc.tile_pool(name="cpool", bufs=1) as cpool,
            tc.tile_pool(name="opool", bufs=1) as opool,
            tc.tile_pool(name="apool", bufs=2) as apool,
            tc.psum_pool(name="psP", bufs=2) as psP,
            tc.psum_pool(name="psQ", bufs=2) as psQ,
        ):
            ident = cpool.tile([P, P], F16)
            nc.sync.dma_start(out=ident, in_=ident_d[:, :])
            w3 = cpool.tile([P, 3], F32)
            nc.sync.dma_start(out=w3, in_=w3_d[:, :])

            rs_sb = opool.tile([P, NT, H], F32)
            rm_sb = opool.tile([P, NT, H], F16)
            mom_sb = opool.tile([3, NT, P], F32)

            for rep in range(repeats):
                pend = []  # (tile_idx, Asb)
                for t in range(NT):
                    xb = xpool.tile([P, H, W], F16, tag="x")
                    # chunked loads at the pipeline edges: tile 0 in quarters
                    # (compute ramps up after the first 512KB), last tile in
                    # halves (only a half-tree of folds remains after the
                    # final chunk lands)
                    nchunk = 4 if t == 0 else (2 if t == NT - 1 else 1)
                    hs = H // nchunk
                    for c in range(nchunk):
                        nc.gpsimd.dma_start(
                            out=xb[:, c * hs : (c + 1) * hs, :],
                            in_=x[t * P : (t + 1) * P, c * hs : (c + 1) * hs, :],
                        )
                    xf = xb.rearrange("p a b -> p (a b)")

                    # --- DVE rowsum + rowmax folds over k (fp16 2x mode) ---
                    # contiguous-halves trees; L1 runs per load-chunk on the
                    # edge tiles, and the last tile's whole tree is split by
                    # row-halves to shorten the drain
                    for tag, op, dst in (("sc", Alu.add, rs_sb), ("mc", Alu.max, rm_sb)):
                        sc = fpool.tile([P, H, W // 2], F16, tag=tag)
                        fold_chunks = nchunk if t == NT - 1 else 1
                        fhs = H // fold_chunks
                        for fc in range(fold_chunks):
                            r0, r1 = fc * fhs, (fc + 1) * fhs
                            if nchunk > 1 and fold_chunks == 1:
                                for c in range(nchunk):
                                    nc.vector.tensor_tensor(
                                        out=sc[:, c * hs : (c + 1) * hs, :],
                                        in0=xb[:, c * hs : (c + 1) * hs, 0:32],
                                        in1=xb[:, c * hs : (c + 1) * hs, 32:64], op=op,
                                    )
                            else:
                                nc.vector.tensor_tensor(
                                    out=sc[:, r0:r1, :], in0=xb[:, r0:r1, 0:32],
                                    in1=xb[:, r0:r1, 32:64], op=op,
                                )
                            w = W // 4
                            while w >= 2:
                                nc.vector.tensor_tensor(
                                    out=sc[:, r0:r1, 0:w], in0=sc[:, r0:r1, 0:w],
                                    in1=sc[:, r0:r1, w : 2 * w], op=op,
                                )
                                w //= 2
                            nc.vector.tensor_tensor(
                                out=dst[:, t, r0:r1].unsqueeze(2), in0=sc[:, r0:r1, 0:1],
                                in1=sc[:, r0:r1, 1:2], op=op,
                            )

                    # --- PE colsum pyramid: Pt[(j', k), b] += chunk.T ---
                    Pt = psP.tile([P, P], F32, tag="P")
                    for c in range(H * W // P):
                        nc.tensor.matmul(
                            out=Pt, lhsT=xf[:, c * P : (c + 1) * P], rhs=ident,
                            start=(c == 0), stop=(c == H * W // P - 1),
                        )
                    Asb = apool.tile([P, P], F32, tag="A")
                    nc.scalar.activation(out=Asb, in_=Pt, func=ActF.Copy)

                    # stage-2 for the previous tile keeps PE from stalling on
                    # the ACT drain of this tile's pyramid
                    def flush(tp, Asb_p):
                        Qt = psQ.tile([3, P], F32, tag="Q", name="Qt")
                        nc.tensor.matmul(out=Qt, lhsT=w3, rhs=Asb_p,
                                         start=True, stop=True)
                        nc.scalar.activation(
                            out=mom_sb[:, tp, :], in_=Qt, func=ActF.Copy)
                        # per-tile output DMAs overlap the drain with compute
                        nc.sync.dma_start(out=mom_d[:, tp * P : (tp + 1) * P],
                                          in_=mom_sb[:, tp, :])
                        nc.sync.dma_start(out=rs_d[:, tp * H : (tp + 1) * H],
                                          in_=rs_sb[:, tp, :])
                        nc.sync.dma_start(out=rm_d[:, tp * H : (tp + 1) * H],
                                          in_=rm_sb[:, tp, :])

                    if pend:
                        flush(*pend.pop())
                    pend.append((t, Asb))

                flush(*pend.pop())

    _split_multiwait_instructions(nc)
    return nc


_cache: dict[int, bass.Bass] = {}


def _get(bpc: int) -> bass.Bass:
    if bpc not in _cache:
        _cache[bpc] = build(bpc)
    return _cache[bpc]


def _consts():
    ident = np.eye(P, dtype=np.float16)
    k = (np.arange(P) % W).astype(np.float32)
    w3 = np.stack([np.ones(P, np.float32), k, k * k], axis=1)  # [128, 3]
    return ident, w3


def _prepare(tensor: np.ndarray):
    t = np.ascontiguousarray(np.asarray(tensor), dtype=np.float32)
    bt = t.shape[0]
    bpc = bt // NCORES
    nc = _get(bpc)
    ident, w3 = _consts()
    in_maps = [
        {"x": t[c * bpc : (c + 1) * bpc], "ident": ident, "w3": w3}
        for c in range(NCORES)
    ]
    return nc, in_maps, t


def _postprocess(t: np.ndarray, results: list[dict]) -> np.ndarray:
    bt = t.shape[0]
    bpc = bt // NCORES
    nt = bpc // P

    mom = np.concatenate(
        [r["mom"].reshape(3, bpc) for r in results], axis=1
    ).astype(np.float64)  # [3, B] batch index = c*bpc + t*128 + p
    rs = np.concatenate(
        [r["rs"].reshape(P, nt, H).transpose(1, 0, 2).reshape(bpc, H)
         for r in results], axis=0)  # [B, H] f32, b = c*bpc + t*128 + p
    rm = np.concatenate(
        [r["rm"].reshape(P, nt, H).transpose(1, 0, 2).reshape(bpc, H)
         for r in results], axis=0).astype(np.float32)

    S, Sk, Sk2 = mom[0], mom[1], mom[2]
    j = np.arange(H, dtype=np.float64)
    Sj = rs.astype(np.float64) @ j
    Sj2 = rs.astype(np.float64) @ (j * j)

    # exact first-occurrence argmax: candidates are rows whose bf16 rowmax
    # ties the bf16 batch max (monotone cast -> true argmax row included)
    Mb = rm.max(axis=1)
    bidx, jidx = np.nonzero(rm == Mb[:, None])
    key = t[bidx, jidx, :].max(axis=1)  # f32 row maxima of candidates
    order = np.lexsort((jidx, -key, bidx))  # per batch: max key, then min j
    first = np.searchsorted(bidx[order], np.arange(bt))
    jstar = jidx[order][first]

    rows = t[np.arange(bt), jstar, :]
    kstar = (rows == rows.max(axis=1)[:, None]).argmax(axis=1)

    js = jstar.astype(np.float64)
    ks = kstar.astype(np.float64)
    loss = ((js * js + ks * ks) * S - 2.0 * js * Sj - 2.0 * ks * Sk + Sj2 + Sk2).sum()
    return np.asarray([loss], dtype=np.float32)


def kernel(tensor: np.ndarray) -> np.ndarray:
    nc, in_maps, t = _prepare(tensor)
    res = run_bass_kernel_spmd(nc, in_maps, list(range(NCORES)))
    return _postprocess(t, res.results)
